# revision 1
# baseline (speedup 1.0000x reference)
"""Trainium2 Bass kernel for the multi-view contrastive loss problem.

Sharding: data-parallel over anchor rows of the two similarity matrices
(sup K=4608 rows -> 5 row-tiles/core incl. padding; unsup K=6144 -> 6/core).
Each core computes its [rows, K] Gram block vs the full (replicated) embedding
set, fused exp+row-sum on ScalarE, and masked numerator sums via per-row dot
products (the pos/den masks collapse algebraically: den == 1-eye for both
losses, numerators factor through label-block / sibling-view embedding sums).
Host gathers/normalizes/reshapes (data prep) and sums 8 per-core scalars.
"""
import sys
sys.path.insert(0, "/opt/trn_rl_repo")
import numpy as np

import concourse.bass as bass
import concourse.tile as tile
from concourse import bacc
from contextlib import ExitStack
from concourse import mybir
from concourse.bass_utils import run_bass_kernel_spmd

N, D, V = 20000, 128, 3
TEMP = 0.2
KS, KU = 4608, 6144          # sup/unsup row counts (view-major)
NS_T, NU_T = 5, 6            # row tiles per core (sup padded: 36 -> 40 slots)
CS, CU = KS // 512, KU // 512  # 9, 12 column chunks
E5 = float(np.exp(5.0))
NB = 157                     # bce free size: 157*128 = 20096 >= 20000
F32 = mybir.dt.float32
F32R = mybir.dt.float32r

_CACHED = {}


def _build_module():
    nc = bacc.Bacc("TRN2", target_bir_lowering=False, debug=False)

    def din(name, shape):
        return nc.dram_tensor(name, shape, F32, kind="ExternalInput").ap()

    znsT = din("znsT", [128, KS])
    znuT = din("znuT", [128, KU])
    zsrT = din("zsrT", [128, NS_T * 128])
    zurT = din("zurT", [128, NU_T * 128])
    zsr = din("zsr", [128, NS_T * 128])
    zur = din("zur", [128, NU_T * 128])
    zma = din("zma", [128, NU_T * 128])
    zmb = din("zmb", [128, NU_T * 128])
    u1b = din("u1b", [128, 128])
    u0b = din("u0b", [128, 128])
    selm = din("selm", [128, NS_T])
    icntm = din("icntm", [128, NS_T])
    valm = din("valm", [128, NS_T])
    bx = din("bx", [128, NB])
    by = din("by", [128, NB])
    bmk = din("bmk", [128, NB])
    bv = din("bv", [128, 3 * NB])
    res = nc.dram_tensor("res", [1, 16], F32, kind="ExternalOutput").ap()

    AF = mybir.ActivationFunctionType

    with tile.TileContext(nc) as tc, ExitStack() as ctx:
        big = ctx.enter_context(tc.tile_pool(name="big", bufs=1))
        sml = ctx.enter_context(tc.tile_pool(name="sml", bufs=1))
        scr = ctx.enter_context(tc.tile_pool(name="scr", bufs=3))
        psum = ctx.enter_context(tc.tile_pool(name="psum", bufs=6, space="PSUM"))
        pfin = ctx.enter_context(tc.tile_pool(name="pfin", bufs=2, space="PSUM"))

        def load(ap, shape, tag, dt=F32):
            t = big.tile(shape, dt, tag=tag)
            nc.gpsimd.dma_start(t[:], ap.bitcast(dt) if dt is not F32 else ap)
            return t

        s_znsT = load(znsT, [128, KS], "znsT")
        s_znuT = load(znuT, [128, KU], "znuT")
        s_zsrT = load(zsrT, [128, NS_T * 128], "zsrT")
        s_zurT = load(zurT, [128, NU_T * 128], "zurT")
        s_zsr = load(zsr, [128, NS_T * 128], "zsr")
        s_zur = load(zur, [128, NU_T * 128], "zur")
        s_zma = load(zma, [128, NU_T * 128], "zma")
        s_zmb = load(zmb, [128, NU_T * 128], "zmb")
        s_u1b = load(u1b, [128, 128], "u1b")
        s_u0b = load(u0b, [128, 128], "u0b")
        s_selm = load(selm, [128, NS_T], "selm")
        s_icntm = load(icntm, [128, NS_T], "icntm")
        s_valm = load(valm, [128, NS_T], "valm")
        s_bx = load(bx, [128, NB], "bx")
        s_by = load(by, [128, NB], "by")
        s_bmk = load(bmk, [128, NB], "bmk")
        s_bv = load(bv, [128, 3 * NB], "bv")

        # ---------- per-row numerator dot products (row-major layout) ----------
        du1 = sml.tile([128, NS_T], F32)
        du0 = sml.tile([128, NS_T], F32)
        sii = sml.tile([128, NS_T], F32)
        for t in range(NS_T):
            rt = s_zsr[:, t * 128:(t + 1) * 128]
            o = scr.tile([128, 128], F32, tag="ttro")

            nc.vector.tensor_mul(o[:], rt, s_u1b[:])

            nc.vector.tensor_reduce(out=du1[:, t:t + 1], in_=o[:], axis=mybir.AxisListType.X, op=mybir.AluOpType.add)
            o = scr.tile([128, 128], F32, tag="ttro")

            nc.vector.tensor_mul(o[:], rt, s_u0b[:])

            nc.vector.tensor_reduce(out=du0[:, t:t + 1], in_=o[:], axis=mybir.AxisListType.X, op=mybir.AluOpType.add)
            o = scr.tile([128, 128], F32, tag="ttro")

            nc.vector.tensor_mul(o[:], rt, rt)

            nc.vector.tensor_reduce(out=sii[:, t:t + 1], in_=o[:], axis=mybir.AxisListType.X, op=mybir.AluOpType.add)
        tab = sml.tile([128, NU_T], F32)
        tbb = sml.tile([128, NU_T], F32)
        for t in range(NU_T):
            rt = s_zur[:, t * 128:(t + 1) * 128]
            o = scr.tile([128, 128], F32, tag="ttro")
            nc.vector.tensor_mul(o[:], rt, s_zma[:, t * 128:(t + 1) * 128])
            nc.vector.tensor_reduce(out=tab[:, t:t + 1], in_=o[:], axis=mybir.AxisListType.X, op=mybir.AluOpType.add)
            o = scr.tile([128, 128], F32, tag="ttro")
            nc.vector.tensor_mul(o[:], rt, s_zmb[:, t * 128:(t + 1) * 128])
            nc.vector.tensor_reduce(out=tbb[:, t:t + 1], in_=o[:], axis=mybir.AxisListType.X, op=mybir.AluOpType.add)

        # num_s = (du0 + sel*(du1-du0) - sii) * icnt   [icnt = 5/cnt host-side]
        n1 = sml.tile([128, NS_T], F32)
        nc.vector.tensor_sub(n1[:], du1[:], du0[:])
        n2 = sml.tile([128, NS_T], F32)
        nc.vector.tensor_mul(n2[:], n1[:], s_selm[:])
        n3 = sml.tile([128, NS_T], F32)
        nc.vector.tensor_add(n3[:], n2[:], du0[:])
        n4 = sml.tile([128, NS_T], F32)
        nc.vector.tensor_sub(n4[:], n3[:], sii[:])
        nds = sml.tile([128, NS_T], F32)
        nc.vector.tensor_mul(nds[:], n4[:], s_icntm[:])
        # num_u = (ta+tb)*2.5
        n5 = sml.tile([128, NU_T], F32)
        nc.vector.tensor_add(n5[:], tab[:], tbb[:])
        ndu = sml.tile([128, NU_T], F32)
        nc.vector.tensor_scalar_mul(ndu[:], in0=n5[:], scalar1=2.5)

        # ---------- Gram + exp + row-sum (the heavy part) ----------
        dps = sml.tile([128, NS_T * CS], F32)
        dpu = sml.tile([128, NU_T * CU], F32)
        for t in range(NS_T):
            lhsT = s_zsrT[:, t * 128:(t + 1) * 128]
            for c in range(CS):
                g = psum.tile([128, 512], F32, tag="gram")
                nc.tensor.matmul(g[:], lhsT, s_znsT[:, c * 512:(c + 1) * 512],
                                 start=True, stop=True)
                e = scr.tile([128, 512], F32, tag="esc")
                nc.scalar.activation(e[:], g[:], AF.Exp, scale=5.0)
                nc.vector.tensor_reduce(out=dps[:, t * CS + c:t * CS + c + 1], in_=e[:],
                                        axis=mybir.AxisListType.X, op=mybir.AluOpType.add)
        for t in range(NU_T):
            lhsT = s_zurT[:, t * 128:(t + 1) * 128]
            for c in range(CU):
                g = psum.tile([128, 512], F32, tag="gram")
                nc.tensor.matmul(g[:], lhsT, s_znuT[:, c * 512:(c + 1) * 512],
                                 start=True, stop=True)
                e = scr.tile([128, 512], F32, tag="esc")
                nc.scalar.activation(e[:], g[:], AF.Exp, scale=5.0)
                nc.vector.tensor_reduce(out=dpu[:, t * CU + c:t * CU + c + 1], in_=e[:],
                                        axis=mybir.AxisListType.X, op=mybir.AluOpType.add)

        # denom = sum(chunk partials) - e^5 ; logd = ln(denom)
        dens = sml.tile([128, NS_T], F32)
        denu = sml.tile([128, NU_T], F32)
        for t in range(NS_T):
            nc.vector.tensor_reduce(
                out=dens[:, t:t + 1], in_=dps[:, t * CS:(t + 1) * CS],
                axis=mybir.AxisListType.X, op=mybir.AluOpType.add)
        for t in range(NU_T):
            nc.vector.tensor_reduce(
                out=denu[:, t:t + 1], in_=dpu[:, t * CU:(t + 1) * CU],
                axis=mybir.AxisListType.X, op=mybir.AluOpType.add)
        dens2 = sml.tile([128, NS_T], F32)
        nc.vector.tensor_scalar_add(dens2[:], in0=dens[:], scalar1=-E5)
        denu2 = sml.tile([128, NU_T], F32)
        nc.vector.tensor_scalar_add(denu2[:], in0=denu[:], scalar1=-E5)
        logs = sml.tile([128, NS_T], F32)
        logu = sml.tile([128, NU_T], F32)
        nc.scalar.activation(logs[:], dens2[:], AF.Ln)
        nc.scalar.activation(logu[:], denu2[:], AF.Ln)

        # loss rows + partial sums
        lr0 = sml.tile([128, NS_T], F32)
        nc.vector.tensor_sub(lr0[:], logs[:], nds[:])
        lr_s = sml.tile([128, NS_T], F32)
        nc.vector.tensor_mul(lr_s[:], lr0[:], s_valm[:])
        lr_u = sml.tile([128, NU_T], F32)
        nc.vector.tensor_sub(lr_u[:], logu[:], ndu[:])
        stack = sml.tile([128, 8], F32)
        nc.vector.memset(stack[:], 0.0)
        nc.vector.tensor_reduce(out=stack[:, 0:1], in_=lr_s[:],
                                axis=mybir.AxisListType.X, op=mybir.AluOpType.add)
        nc.vector.tensor_reduce(out=stack[:, 1:2], in_=lr_u[:],
                                axis=mybir.AxisListType.X, op=mybir.AluOpType.add)

        # ---------- BCE losses: bce = softplus(x) - x*y ----------
        def bce_masked(xap, col):
            e = scr.tile([128, NB], F32, tag="bces")
            nc.scalar.activation(e[:], xap, AF.Exp)
            sp = scr.tile([128, NB], F32, tag="bcesp")
            nc.scalar.activation(sp[:], e[:], AF.Ln, bias=1.0)
            xy = scr.tile([128, NB], F32, tag="bcexy")
            nc.vector.tensor_mul(xy[:], xap, s_by[:])
            d = scr.tile([128, NB], F32, tag="bced")
            nc.vector.tensor_sub(d[:], sp[:], xy[:])
            o = scr.tile([128, NB], F32, tag="bceo")

            nc.vector.tensor_mul(o[:], d[:], s_bmk[:])

            nc.vector.tensor_reduce(out=stack[:, col:col + 1], in_=o[:], axis=mybir.AxisListType.X, op=mybir.AluOpType.add)

        bce_masked(s_bx[:], 2)
        vparts = sml.tile([128, 3], F32)
        for v in range(3):
            e = scr.tile([128, NB], F32, tag="bces")
            nc.scalar.activation(e[:], s_bv[:, v * NB:(v + 1) * NB], AF.Exp)
            sp = scr.tile([128, NB], F32, tag="bcesp")
            nc.scalar.activation(sp[:], e[:], AF.Ln, bias=1.0)
            xy = scr.tile([128, NB], F32, tag="bcexy")
            nc.vector.tensor_mul(xy[:], s_bv[:, v * NB:(v + 1) * NB], s_by[:])
            d = scr.tile([128, NB], F32, tag="bced")
            nc.vector.tensor_sub(d[:], sp[:], xy[:])
            o = scr.tile([128, NB], F32, tag="bceo")

            nc.vector.tensor_mul(o[:], d[:], s_bmk[:])

            nc.vector.tensor_reduce(out=vparts[:, v:v + 1], in_=o[:], axis=mybir.AxisListType.X, op=mybir.AluOpType.add)
        nc.vector.tensor_reduce(out=stack[:, 3:4], in_=vparts[:],
                                axis=mybir.AxisListType.X, op=mybir.AluOpType.add)
        nc.vector.tensor_reduce(out=stack[:, 4:5], in_=s_bmk[:],
                                axis=mybir.AxisListType.X, op=mybir.AluOpType.add)

        # ---------- cross-partition reduction: ones-matmul (fp32, exact) ----------
        ones = sml.tile([128, 1], F32)
        nc.vector.memset(ones[:], 1.0)
        fin = pfin.tile([1, 8], F32)
        nc.tensor.matmul(fin[:], ones[:], stack[:], start=True, stop=True)
        osb = sml.tile([1, 16], F32)
        nc.vector.memset(osb[:], 0.0)
        nc.vector.tensor_copy(osb[:, 0:8], fin[:])
        nc.gpsimd.dma_start(res, osb[:])

    nc.compile()
    return nc


def _prep(inputs):
    proj = np.asarray(inputs["proj"], dtype=np.float32)
    lab_idx = np.concatenate([np.asarray(inputs["train_pos_idx"]),
                              np.asarray(inputs["train_neg_idx"])]).astype(np.int64)
    uidx = np.asarray(inputs["unlabeled_idx"]).astype(np.int64)

    # view-major gather + normalize (float32, matching reference math)
    zs = proj[:, lab_idx].reshape(KS, D)
    zu = proj[:, uidx].reshape(KU, D)

    def norm(z):
        n = np.sqrt((z.astype(np.float64) ** 2).sum(-1, keepdims=True))
        return (z / np.maximum(n, 1e-8)).astype(np.float32)

    zns, znu = norm(zs), norm(zu)
    lab_vm = np.tile(np.concatenate([np.ones(512), np.zeros(1024)]), V)
    u1 = zns[lab_vm == 1].sum(0, dtype=np.float64).astype(np.float32)
    u0 = zns[lab_vm == 0].sum(0, dtype=np.float64).astype(np.float32)

    znu3 = znu.reshape(V, KU // V, D)
    sibs = {0: (1, 2), 1: (0, 2), 2: (0, 1)}

    def pad_bce(a):
        out = np.zeros(NB * 128, np.float32)
        out[:N] = a
        return out.reshape(NB, 128).T.copy()

    bx = pad_bce(np.asarray(inputs["fused_logit"], np.float32))
    by = pad_bce(np.asarray(inputs["labels"], np.float32))
    bmk = pad_bce(np.asarray(inputs["train_mask"]).astype(np.float32))
    vl = np.asarray(inputs["view_logits"], np.float32)
    bv = np.hstack([pad_bce(vl[v]) for v in range(3)])

    common = dict(
        znsT=np.ascontiguousarray(zns.T), znuT=np.ascontiguousarray(znu.T),
        u1b=np.broadcast_to(u1, (128, 128)).copy(),
        u0b=np.broadcast_to(u0, (128, 128)).copy(),
        bx=bx, by=by, bmk=bmk, bv=bv,
    )

    in_maps = []
    for c in range(8):
        sup_tiles = list(range(5 * c, 5 * c + 5)) if c < 4 else \
            list(range(20 + 4 * (c - 4), 24 + 4 * (c - 4))) + [-1]
        zsr_t = np.zeros((NS_T, 128, D), np.float32)
        selm = np.zeros((128, NS_T), np.float32)
        icntm = np.ones((128, NS_T), np.float32)
        valm = np.zeros((128, NS_T), np.float32)
        for k, g in enumerate(sup_tiles):
            if g < 0:
                zsr_t[k] = 1.0 / np.sqrt(np.float32(D))  # normalized dummy rows
                continue
            zsr_t[k] = zns[128 * g:128 * (g + 1)]
            r = 128 * g + np.arange(128)
            sel = (r % 1536) < 512
            selm[:, k] = sel
            icntm[:, k] = 5.0 / np.where(sel, 1535.0, 3071.0)
            valm[:, k] = 1.0
        ut0 = 6 * c
        zur_t = znu[128 * ut0:128 * (ut0 + 6)].reshape(NU_T, 128, D)
        r = (128 * ut0 + np.arange(NU_T * 128))
        m, a = r // (KU // V), r % (KU // V)
        sa = np.array([sibs[mm][0] for mm in m])
        sb = np.array([sibs[mm][1] for mm in m])
        zma_t = znu3[sa, a].reshape(NU_T, 128, D)
        zmb_t = znu3[sb, a].reshape(NU_T, 128, D)

        def rowmajor(zt):  # [T,128,D] -> [128, T*128] with tile t at cols t*128..
            return np.ascontiguousarray(zt.transpose(1, 0, 2).reshape(128, NU_T * D)
                                        if zt.shape[0] == NU_T else
                                        zt.transpose(1, 0, 2).reshape(128, NS_T * D))

        # stationary: columns grouped per tile: [:, t*128+p] = row p of tile t
        zsrT = np.ascontiguousarray(np.concatenate([zsr_t[t].T for t in range(NS_T)], axis=1))
        zurT = np.ascontiguousarray(np.concatenate([zur_t[t].T for t in range(NU_T)], axis=1))

        in_maps.append(dict(
            common,
            zsrT=zsrT, zurT=zurT,
            zsr=rowmajor(zsr_t), zur=rowmajor(zur_t),
            zma=rowmajor(zma_t), zmb=rowmajor(zmb_t),
            selm=selm, icntm=icntm, valm=valm,
        ))
    return in_maps


def kernel(**inputs):
    if "nc" not in _CACHED:
        _CACHED["nc"] = _build_module()
    nc = _CACHED["nc"]
    in_maps = _prep(inputs)
    r = run_bass_kernel_spmd(nc, in_maps, core_ids=list(range(8)))
    outs = [r.results[c]["res"][0] for c in range(8)]
    sup = sum(float(o[0]) for o in outs) / KS
    unsup = sum(float(o[1]) for o in outs) / KU
    o0 = outs[0]
    msum = max(float(o0[4]), 1.0)
    main = float(o0[2]) / msum
    view = float(o0[3]) / (3.0 * msum)
    total = main + view + sup + 0.2 * unsup
    return np.array([total, main, view, sup, unsup], dtype=np.float32)

# test harness hook: stash last run results (exec_time_ns etc.)
_orig_run = run_bass_kernel_spmd
def run_bass_kernel_spmd(nc, in_maps, core_ids, **kw):  # noqa: F811
    r = _orig_run(nc, in_maps, core_ids, **kw)
    _CACHED["last_r"] = r
    return r



# revision 2
# speedup vs baseline: 1.5232x; 1.5232x over previous
"""Trainium2 Bass kernel for the multi-view contrastive loss problem.

Row-sharded over the 10752 combined anchor rows (sup 4608 + unsup 6144) of
the two similarity matrices: core c owns combined rows [1344c, 1344(c+1)),
processed as 12 tiles of 112 rows. Each core receives ONLY its 1/8 shard of
the normalized embeddings (fp8) in a single packed input tensor; the full
column set is reassembled on-device via AllGather over the device
interconnect. Numerators are computed as on-device dot products against a
shipped per-row "numerator matrix" W (label-class embedding sums for sup
rows, sibling-view sums for unsup rows), so no similarity-matrix masks or
row-major embedding copies are ever shipped. The BCE terms are sharded
elementwise across cores. Host sums 8 per-core scalar partials.

Everything rides in one [128, 780] int32 tensor per core (~400 KB, ~3.2 MB
total vs 67 MB for the replicated layout), which matters because the axon
tunnel moves ~43 MB/s.
"""
import sys
sys.path.insert(0, "/opt/trn_rl_repo")
import numpy as np
import ml_dtypes

import concourse.bass as bass
import concourse.tile as tile
from concourse import bacc
from contextlib import ExitStack
from concourse import mybir

N, D, V = 20000, 128, 3
KS, KU = 4608, 6144           # sup/unsup row counts (view-major)
KT = KS + KU                  # 10752 combined embedding columns
NCORE = 8
ZC = KT // NCORE              # 1344 combined cols (rows) per core
RT = 112                      # row-tile height; 12 tiles * 112 = 1344
NT = ZC // RT                 # 12 row tiles per core
CS, CU = KS // 512, KU // 512  # 9 sup + 12 unsup column chunks
NC_CH = CS + CU               # 21
E5 = float(np.exp(5.0))
NB = 20                       # bce cols per core: 20*128=2560 >= 2500
NBE = N // NCORE              # 2500 bce elements per core
F32 = mybir.dt.float32
BF16 = mybir.dt.bfloat16
F8 = mybir.dt.float8e4
I32 = mybir.dt.int32

# packed input layout, in int32-sized columns
PK_Z = ZC // 4                # 336: fp8 embedding shard [128, 1344]
PK_W = ZC // 4                # 336: fp8 numerator matrix shard [128, 1344]
PK_B = 6 * NB // 2            # 60: bf16 bce planes (x, y, m, v0, v1, v2)
PK_M = 4 * NT                 # 48: f32 masks (is_sup, is_unsup, nscale, cadd)
PKW = PK_Z + PK_W + PK_B + PK_M  # 780
O_W = PK_Z
O_B = PK_Z + PK_W
O_M = PK_Z + PK_W + PK_B

_CACHED = {}


def _build_module():
    nc = bacc.Bacc("TRN2", target_bir_lowering=False, debug=False,
                   num_devices=NCORE)
    pk = nc.dram_tensor("pk", [128, PKW], I32, kind="ExternalInput").ap()
    res = nc.dram_tensor("res", [1, 16], F32, kind="ExternalOutput").ap()
    AF = mybir.ActivationFunctionType

    with tile.TileContext(nc) as tc, ExitStack() as ctx:
        big = ctx.enter_context(tc.tile_pool(name="big", bufs=1))
        sml = ctx.enter_context(tc.tile_pool(name="sml", bufs=1))
        scr = ctx.enter_context(tc.tile_pool(name="scr", bufs=3))
        psum = ctx.enter_context(tc.tile_pool(name="psum", bufs=4, space="PSUM"))
        psum2 = ctx.enter_context(tc.tile_pool(name="psum2", bufs=2, space="PSUM"))
        pfin = ctx.enter_context(tc.tile_pool(name="pfin", bufs=1, space="PSUM"))
        dram = ctx.enter_context(tc.tile_pool(name="dram", bufs=2, space="DRAM"))

        # ---- AllGather the fp8 embedding shards (DRAM->DRAM) ----
        in_b = dram.tile([128, ZC], F8)
        out_b = dram.tile([NCORE * 128, ZC], F8)
        nc.gpsimd.dma_start(in_b[:], pk[:, 0:PK_Z].bitcast(F8))
        nc.gpsimd.collective_compute(
            "AllGather", mybir.AluOpType.bypass,
            replica_groups=[list(range(NCORE))],
            ins=[in_b.opt()], outs=[out_b.opt()],
        )
        s_z = big.tile([128, KT], F8, tag="zall")
        for c in range(NCORE):
            nc.gpsimd.dma_start(s_z[:, c * ZC:(c + 1) * ZC],
                                out_b[c * 128:(c + 1) * 128, :])

        # ---- per-core inputs ----
        s_own = sml.tile([128, ZC], F8)
        nc.gpsimd.dma_start(s_own[:], pk[:, 0:PK_Z].bitcast(F8))
        s_w = sml.tile([128, ZC], F8)
        nc.gpsimd.dma_start(s_w[:], pk[:, O_W:O_W + PK_W].bitcast(F8))
        s_bce16 = sml.tile([128, 6 * NB], BF16)
        nc.gpsimd.dma_start(s_bce16[:], pk[:, O_B:O_B + PK_B].bitcast(BF16))
        s_msk = sml.tile([128, PK_M], F32)
        nc.gpsimd.dma_start(s_msk[:], pk[:, O_M:PKW].bitcast(F32))
        m_sup = s_msk[:, 0:NT]
        m_uns = s_msk[:, NT:2 * NT]
        m_nsc = s_msk[:, 2 * NT:3 * NT]
        m_cad = s_msk[:, 3 * NT:4 * NT]

        # identity mask for diagonal extraction
        eye = sml.tile([128, 128], F32)
        nc.vector.memset(eye[:], 1.0)
        nc.gpsimd.affine_select(eye[:], eye[:], pattern=[[-1, 128]],
                                compare_op=mybir.AluOpType.is_equal, fill=0.0,
                                base=0, channel_multiplier=1)

        den_s = sml.tile([128, NT], F32)
        den_u = sml.tile([128, NT], F32)
        num = sml.tile([128, NT], F32)
        nc.vector.memset(den_s[:], 0.0)
        nc.vector.memset(den_u[:], 0.0)
        nc.vector.memset(num[:], 0.0)

        # ---- Gram + exp + row-sums, and numerator dots (the heavy part) ----
        for t in range(NT):
            lhsT = s_own[:, t * RT:(t + 1) * RT]
            # numerator: diag( own_tile^T @ W_tile )
            g2 = psum2.tile([RT, RT], F32, tag="g2")
            nc.tensor.matmul(g2[:], lhsT, s_w[:, t * RT:(t + 1) * RT],
                             start=True, stop=True)
            o2 = scr.tile([RT, RT], F32, tag="o2")
            nc.vector.tensor_mul(o2[:], g2[:], eye[0:RT, 0:RT])
            nc.vector.tensor_reduce(out=num[0:RT, t:t + 1], in_=o2[:],
                                    axis=mybir.AxisListType.X,
                                    op=mybir.AluOpType.add)
            # denominators: exp row-sums over all 21 column chunks
            dsc = scr.tile([128, NC_CH], F32, tag="dsc")
            for k in range(NC_CH):
                g = psum.tile([RT, 512], F32, tag="gram")
                nc.tensor.matmul(g[:], lhsT, s_z[:, k * 512:(k + 1) * 512],
                                 start=True, stop=True)
                e = scr.tile([RT, 512], F32, tag="esc")
                nc.scalar.activation(e[:], g[:], AF.Exp, scale=5.0)
                nc.vector.tensor_reduce(out=dsc[0:RT, k:k + 1], in_=e[:],
                                        axis=mybir.AxisListType.X,
                                        op=mybir.AluOpType.add)
            nc.vector.tensor_reduce(out=den_s[0:RT, t:t + 1],
                                    in_=dsc[0:RT, 0:CS],
                                    axis=mybir.AxisListType.X,
                                    op=mybir.AluOpType.add)
            nc.vector.tensor_reduce(out=den_u[0:RT, t:t + 1],
                                    in_=dsc[0:RT, CS:NC_CH],
                                    axis=mybir.AxisListType.X,
                                    op=mybir.AluOpType.add)

        # ---- per-row losses ----
        def log_den(den):
            d1 = sml.tile([128, NT], F32)
            nc.vector.tensor_scalar_add(d1[:], in0=den[:], scalar1=-E5)
            d2 = sml.tile([128, NT], F32)
            nc.vector.tensor_scalar_max(d2[:], in0=d1[:], scalar1=1.0)
            lg = sml.tile([128, NT], F32)
            nc.scalar.activation(lg[:], d2[:], AF.Ln)
            return lg

        log_s = log_den(den_s)
        log_u = log_den(den_u)
        nsc = sml.tile([128, NT], F32)
        nc.vector.tensor_mul(nsc[:], num[:], m_nsc)
        base = sml.tile([128, NT], F32)
        nc.vector.tensor_sub(base[:], m_cad, nsc[:])   # cadd - num*nscale

        stack = sml.tile([128, 8], F32)
        nc.vector.memset(stack[:], 0.0)

        def loss_part(lg, mask, col):
            a = sml.tile([128, NT], F32)
            nc.vector.tensor_add(a[:], lg[:], base[:])
            b = sml.tile([128, NT], F32)
            nc.vector.tensor_mul(b[:], a[:], mask)
            nc.vector.tensor_reduce(out=stack[:, col:col + 1], in_=b[:],
                                    axis=mybir.AxisListType.X,
                                    op=mybir.AluOpType.add)

        loss_part(log_s, m_sup, 0)
        loss_part(log_u, m_uns, 1)

        # ---- BCE losses (sharded elementwise): bce = ln(1+e^x) - x*y ----
        s_bce = sml.tile([128, 6 * NB], F32)
        nc.vector.tensor_copy(s_bce[:], s_bce16[:])
        p_y = s_bce[:, NB:2 * NB]
        p_m = s_bce[:, 2 * NB:3 * NB]

        def bce_to(xap, outap):
            e = scr.tile([128, NB], F32, tag="bces")
            nc.scalar.activation(e[:], xap, AF.Exp)
            sp = scr.tile([128, NB], F32, tag="bcesp")
            nc.scalar.activation(sp[:], e[:], AF.Ln, bias=1.0)
            xy = scr.tile([128, NB], F32, tag="bcexy")
            nc.vector.tensor_mul(xy[:], xap, p_y)
            d = scr.tile([128, NB], F32, tag="bced")
            nc.vector.tensor_sub(d[:], sp[:], xy[:])
            o = scr.tile([128, NB], F32, tag="bceo")
            nc.vector.tensor_mul(o[:], d[:], p_m)
            nc.vector.tensor_reduce(out=outap, in_=o[:],
                                    axis=mybir.AxisListType.X,
                                    op=mybir.AluOpType.add)

        bce_to(s_bce[:, 0:NB], stack[:, 2:3])
        vparts = sml.tile([128, 3], F32)
        for v in range(3):
            bce_to(s_bce[:, (3 + v) * NB:(4 + v) * NB], vparts[:, v:v + 1])
        nc.vector.tensor_reduce(out=stack[:, 3:4], in_=vparts[:],
                                axis=mybir.AxisListType.X,
                                op=mybir.AluOpType.add)
        nc.vector.tensor_reduce(out=stack[:, 4:5], in_=p_m,
                                axis=mybir.AxisListType.X,
                                op=mybir.AluOpType.add)

        # ---- cross-partition reduction: ones-matmul (fp32, exact) ----
        ones = sml.tile([128, 1], F32)
        nc.vector.memset(ones[:], 1.0)
        fin = pfin.tile([1, 8], F32)
        nc.tensor.matmul(fin[:], ones[:], stack[:], start=True, stop=True)
        osb = sml.tile([1, 16], F32)
        nc.vector.memset(osb[:], 0.0)
        nc.vector.tensor_copy(osb[:, 0:8], fin[:])
        nc.gpsimd.dma_start(res, osb[:])

    nc.compile()
    return nc


def _static_parts():
    """Input-independent pieces: per-core mask planes + bce padding map."""
    r = np.arange(KT)                      # combined col index
    is_sup = (r < KS).astype(np.float32)
    i_lab = r % 1536                       # within-view sup index (valid where sup)
    sel = (i_lab < 512)
    cnt = np.where(sel, 1535.0, 3071.0)
    icnt = (5.0 / cnt).astype(np.float32)
    nscale = np.where(r < KS, icnt, 2.5).astype(np.float32)
    cadd = np.where(r < KS, icnt, 0.0).astype(np.float32)

    mask_blocks = np.zeros((NCORE, 128, PK_M), np.float32)
    for c in range(NCORE):
        for t in range(NT):
            rr = ZC * c + RT * t + np.arange(RT)
            mask_blocks[c, 0:RT, t] = is_sup[rr]
            mask_blocks[c, 0:RT, NT + t] = 1.0 - is_sup[rr]
            mask_blocks[c, 0:RT, 2 * NT + t] = nscale[rr]
            mask_blocks[c, 0:RT, 3 * NT + t] = cadd[rr]
    return mask_blocks.view(np.int32)


def _prep(inputs):
    proj = np.asarray(inputs["proj"], dtype=np.float32)
    lab_idx = np.concatenate([np.asarray(inputs["train_pos_idx"]),
                              np.asarray(inputs["train_neg_idx"])]).astype(np.int64)
    uidx = np.asarray(inputs["unlabeled_idx"]).astype(np.int64)

    # view-major gather + fp32 normalize (matching reference math)
    zs = proj[:, lab_idx].reshape(KS, D)
    zu = proj[:, uidx].reshape(KU, D)
    zn = np.concatenate([zs, zu], axis=0)
    nrm = np.sqrt(np.einsum("ij,ij->i", zn, zn, dtype=np.float64))
    zn /= np.maximum(nrm, 1e-8).astype(np.float32)[:, None]

    # numerator matrix W: label-class sums (sup) / sibling-view sums (unsup)
    zns = zn[:KS]
    u1 = zns.reshape(V, 1536, D)[:, :512].sum(axis=(0, 1), dtype=np.float64)
    u0 = zns.reshape(V, 1536, D)[:, 512:].sum(axis=(0, 1), dtype=np.float64)
    w = np.empty((KT, D), np.float32)
    sel = (np.arange(KS) % 1536) < 512
    w[:KS] = np.where(sel[:, None], u1[None, :], u0[None, :])
    znu3 = zn[KS:].reshape(V, 2048, D)
    msum = znu3[[1, 2, 0]] + znu3[[2, 0, 1]]   # siblings of view v
    w[KS:] = msum.reshape(KU, D)

    z8T = np.ascontiguousarray(zn.astype(ml_dtypes.float8_e4m3).T)
    w8T = np.ascontiguousarray(w.astype(ml_dtypes.float8_e4m3).T)

    # bce planes, bf16, elementwise-sharded: core c owns [2500c, 2500(c+1))
    bcef = np.zeros((6, NCORE * NB * 128), ml_dtypes.bfloat16)
    bcef[0, :N] = np.asarray(inputs["fused_logit"], np.float32)
    bcef[1, :N] = np.asarray(inputs["labels"], np.float32)
    bcef[2, :N] = np.asarray(inputs["train_mask"]).astype(np.float32)
    vl = np.asarray(inputs["view_logits"], np.float32)
    for v in range(3):
        bcef[3 + v, :N] = vl[v]
    # [6, NCORE, NB, 128] -> per-core planes [128, 6*NB]
    bplanes = bcef.reshape(6, NCORE, NB, 128).transpose(1, 3, 0, 2)

    if "masks" not in _CACHED:
        _CACHED["masks"] = _static_parts()
    masks = _CACHED["masks"]

    big = np.empty((NCORE * 128, PKW), np.int32)
    for c in range(NCORE):
        blk = big[c * 128:(c + 1) * 128]
        blk[:, 0:PK_Z] = np.ascontiguousarray(
            z8T[:, c * ZC:(c + 1) * ZC]).view(np.int32)
        blk[:, O_W:O_W + PK_W] = np.ascontiguousarray(
            w8T[:, c * ZC:(c + 1) * ZC]).view(np.int32)
        blk[:, O_B:O_B + PK_B] = np.ascontiguousarray(
            bplanes[c]).reshape(128, 6 * NB).view(np.int32)
        blk[:, O_M:PKW] = masks[c]
    return big


def _get_runner():
    if "run" in _CACHED:
        return _CACHED["run"]
    import jax
    from jax.sharding import Mesh, PartitionSpec
    from jax.experimental.shard_map import shard_map
    from concourse import bass2jax
    from concourse.bass2jax import _bass_exec_p, partition_id_tensor, \
        install_neuronx_cc_hook

    nc = _build_module()
    install_neuronx_cc_hook()
    assert nc.dbg_addr is None or not nc.dbg_callbacks

    partition_name = (nc.partition_id_tensor.name
                      if nc.partition_id_tensor else None)
    dbg_name = nc.dbg_addr.name if nc.dbg_addr is not None else None
    in_names, out_names, out_avals, zero_shapes = [], [], [], []
    for alloc in nc.m.functions[0].allocations:
        if not isinstance(alloc, mybir.MemoryLocationSet):
            continue
        name = alloc.memorylocations[0].name
        if alloc.kind == "ExternalInput":
            if name != partition_name:
                in_names.append(name)
        elif alloc.kind == "ExternalOutput":
            shape = tuple(alloc.tensor_shape)
            dtype = mybir.dt.np(alloc.dtype)
            out_names.append(name)
            out_avals.append(jax.core.ShapedArray(shape, dtype))
            zero_shapes.append((shape, dtype))
    n_params = len(in_names)
    n_outs = len(out_avals)
    in_names_all = in_names + out_names + (
        [partition_name] if partition_name else [])
    donate = tuple(range(n_params, n_params + n_outs))

    def _body(*args):
        operands = list(args)
        if partition_name is not None:
            operands.append(partition_id_tensor())
        outs = _bass_exec_p.bind(
            *operands, out_avals=tuple(out_avals),
            in_names=tuple(in_names_all), out_names=tuple(out_names),
            lowering_input_output_aliases=(), sim_require_finite=True,
            sim_require_nnan=True, nc=nc)
        return tuple(outs)

    devices = jax.devices()[:NCORE]
    mesh = Mesh(np.asarray(devices), ("core",))
    in_specs = (PartitionSpec("core"),) * (n_params + n_outs)
    out_specs = (PartitionSpec("core"),) * len(out_names)
    sharded = jax.jit(shard_map(_body, mesh=mesh, in_specs=in_specs,
                                out_specs=out_specs, check_rep=False),
                      donate_argnums=donate, keep_unused=True)
    order = {n: i for i, n in enumerate(in_names)}
    assert order == {"pk": 0} and out_names == ["res"], (order, out_names)

    def run(big):
        zeros = [np.zeros((NCORE * s[0], *s[1:]), dt) for s, dt in zero_shapes]
        out = sharded(big, *zeros)
        return np.asarray(out[0]).reshape(NCORE, 16)

    _CACHED["run"] = run
    return run


def kernel(**inputs):
    run = _get_runner()
    big = _prep(inputs)
    outs = run(big)
    sup = float(outs[:, 0].sum()) / KS
    unsup = float(outs[:, 1].sum()) / KU
    msum = max(float(outs[:, 4].sum()), 1.0)
    main = float(outs[:, 2].sum()) / msum
    view = float(outs[:, 3].sum()) / (3.0 * msum)
    total = main + view + sup + 0.2 * unsup
    return np.array([total, main, view, sup, unsup], dtype=np.float32)
